# revision 11
# baseline (speedup 1.0000x reference)
"""Multi-head self-attention (B=4, L=1024, D=1024, H=16) on 8 TRN2 NeuronCores.

Sharding: core c handles batch b=c//2 and head-group g=c%2 (8 of 16 heads,
i.e. 512 of 1024 output columns). No collectives needed — each core computes
a disjoint [1024, 512] slice of the output.

Per-core dataflow (all layouts chosen so no on-device transpose is needed):
  - host passes seq^T [D, L], W^T slices [D, 512] (q-weights pre-scaled by
    1/sqrt(dh)), and the combined additive mask transposed: maskT[l_k, l_q]
  - qT/kT [512(j), L]: j-major projections (PE matmul, PSUM accum over D,
    bias added per-partition by ScalarE on the PSUM->SBUF copy)
  - v' [L, 8*(64+1)]: natural-layout V with a ones-column per head (the
    ones-column makes the attn@V matmul also produce the softmax denominator)
  - scoresT_h[l_k, l_q] = kT_h.T @ qT_h  (contraction over dh on partitions)
  - masked blocks are classified at build time from the actual mask values:
    all-masked blocks are skipped entirely, all-zero blocks skip the DVE add,
    partial blocks get a DVE tensor-tensor add of the mask tile
  - softmax without max-subtraction (scores are O(1) for sane inputs; the
    additive mask is clamped to -100 on host): E = exp(scoresT) via ScalarE
  - out'^T_h[dh+1, l_q] = v'_h.T @ E_h accumulated over l_k tiles; row 64 is
    the softmax denominator; DVE multiplies rows 0..63 by its broadcast
    reciprocal -> outT [512(j), L] stored j-major; host transposes on gather.

Matmul-facing tensors are bf16 (full-rate PE, FWL weight loads); all
accumulation is fp32 in PSUM, masks/normalization/output stay fp32.
"""

import sys
import types

import numpy as np

B, L, D, H = 4, 1024, 1024, 16
DH = D // H  # 64
NCORES = 8
HPC = H // 2  # heads per core
JW = HPC * DH  # 512 output cols per core
P = 128
CH = 512  # l_q chunk width
NCH = L // CH  # 2
KT = D // P  # 8 contraction tiles
LT = L // P  # 8 l tiles
JT = JW // P  # 4 j tiles
SCALE = 1.0 / np.sqrt(np.float32(DH))

_MASK_CLAMP = -100.0
_MASK_SKIP = -50.0


def _install_patches():
    """Environment glue for running bass under axon in this container."""
    # 1. antenv.axon_hooks shim so trace=True can reach the NTFF profiler.
    if "antenv.axon_hooks" not in sys.modules:
        try:
            import antenv
            from trn_agent_boot.trn_boot import _ntff_profile_via_ctypes

            hook = _ntff_profile_via_ctypes("/opt/axon/libaxon_pjrt.so")
            mod = types.ModuleType("antenv.axon_hooks")
            _h = [hook]
            mod.set_axon_ntff_profile_hook = lambda h: _h.__setitem__(0, h)
            mod.get_axon_ntff_profile_hook = lambda: _h[0]
            sys.modules["antenv.axon_hooks"] = mod
            antenv.axon_hooks = mod
        except Exception:
            pass

    # 2. no-op artifact upload (no bucket in this sandbox)
    import concourse.bass_utils as bu

    bu.upload_artifacts = lambda tmpdir: tmpdir

    # 2b. this toolchain's walrus encodes at most ONE sync-wait per
    # instruction; split extra waits onto NoOps inserted just before the
    # instruction (same engine => identical blocking semantics).
    import json

    import concourse.bass as bass_mod

    if not getattr(bass_mod.Bass, "_wait_split_patched", False):
        _orig_to_json = bass_mod.Bass.to_json_bytes

        def _split_waits_json(bir_bytes):
            j = json.loads(bir_bytes)
            ctr = 0
            for f in j["functions"]:
                for bb in f["blocks"]:
                    new = []
                    changed = False
                    for inst in bb["instructions"]:
                        si = inst.get("sync_info")
                        waits = si.get("on_wait", []) if si else []
                        if len(waits) > 1 and inst.get("engine", "Unassigned") != "Unassigned":
                            for w in waits[:-1]:
                                ctr += 1
                                new.append(
                                    {
                                        "debug": inst.get("debug", 0),
                                        "engine": inst["engine"],
                                        "ins": [],
                                        "outs": [],
                                        "name": f"I-wsplit-{ctr}",
                                        "opcode": "NoOp",
                                        "sync_info": {"on_update": [], "on_wait": [w]},
                                        "text_hint": "wait_split",
                                    }
                                )
                            si["on_wait"] = [waits[-1]]
                            changed = True
                        new.append(inst)
                    if changed:
                        bb["instructions"] = new
            return json.dumps(j).encode()

        def _to_json_bytes_split(self):
            return _split_waits_json(_orig_to_json(self))

        bass_mod.Bass.to_json_bytes = _to_json_bytes_split
        bass_mod.Bass._wait_split_patched = True

    # 3. split the Tile kernel-tail drain's multi-sem wait into single-wait
    # NOPs (this toolchain's walrus encodes only one sync-wait per CTRL).
    import concourse.tile as tile
    from concourse import mybir
    from concourse.tile import ScopedClock

    if getattr(tile.TileContext, "_drain_split_patched", False):
        return

    def _drain_and_barrier_split(self, tick_clock, wait_clock):
        nc = self.nc
        probe = nc.sync.nop(nofuse=True, hint="drain_wait_probe")
        wait_clock.add_sem_waits(
            probe.ins, ScopedClock({None: tick_clock.global_clock})
        )
        si = probe.ins.sync_info
        waits = list(si.on_wait) if si and si.on_wait else []
        if len(waits) > 1:
            si.on_wait = [waits[0]]
            for w in waits[1:]:
                extra = nc.sync.nop(nofuse=True, hint="drain_wait_split")
                xsi = extra.ins.sync_info
                if xsi is None:
                    extra.ins.sync_info = mybir.SyncInfo(on_wait=[w], on_update=[])
                else:
                    xsi.on_wait = [w]
        nc.sync.drain()
        nc.all_engine_barrier()
        assert self.sems is not None
        popped = nc._tile_sem_poison_stack.pop()
        assert popped is self._sem_poison
        nc.clear_and_free_semaphores(list(self.sems.allocated().values()))
        nc.all_engine_barrier()

    tile.TileContext._drain_and_barrier = _drain_and_barrier_split
    tile.TileContext._drain_split_patched = True


def _prepare_inputs(sequence, padding_mask, dependency_mask, Wq, bq, Wk, bk, Wv, bv):
    """Shard + pre-transpose on host. Returns (in_maps, block classification)."""
    seq = np.asarray(sequence, np.float32)
    pad = np.asarray(padding_mask, np.float32)
    dep = np.asarray(dependency_mask, np.float32)[0, 0]  # [L(q), L(k)]
    depT = np.ascontiguousarray(dep.T)  # [L(k), L(q)]
    Wq = np.asarray(Wq, np.float32)
    Wk = np.asarray(Wk, np.float32)
    Wv = np.asarray(Wv, np.float32)
    bq = np.asarray(bq, np.float32)
    bk = np.asarray(bk, np.float32)
    bv = np.asarray(bv, np.float32)

    seqT = [np.ascontiguousarray(seq[b].T) for b in range(B)]
    maskT = []
    for b in range(B):
        m = depT + pad[b, 0, 0, :][:, None]
        np.clip(m, _MASK_CLAMP, None, out=m)
        maskT.append(m)

    import ml_dtypes

    b16 = ml_dtypes.bfloat16
    in_maps = []
    for c in range(NCORES):
        b, g = c // 2, c % 2
        Jl = g * JW
        in_maps.append(
            {
                "seqT": seqT[b].astype(b16),
                "wqT": np.ascontiguousarray(Wq[Jl : Jl + JW].T * SCALE).astype(b16),
                "wkT": np.ascontiguousarray(Wk[Jl : Jl + JW].T).astype(b16),
                "wvT": np.ascontiguousarray(Wv[Jl : Jl + JW].T).astype(b16),
                "bq": np.ascontiguousarray(bq[Jl : Jl + JW] * SCALE),
                "bk": np.ascontiguousarray(bk[Jl : Jl + JW]),
                "bv": np.ascontiguousarray(bv[Jl : Jl + JW]).astype(b16),
                "maskT": maskT[b],
            }
        )

    # Block classification, shared across cores (the NEFF is SPMD): a block
    # may be skipped only if fully masked for every core; needs a DVE add
    # unless it is exactly zero for every core.
    cls = {}
    for kb in range(LT):
        for c in range(NCH):
            blks = [m[kb * P : (kb + 1) * P, c * CH : (c + 1) * CH] for m in maskT]
            if all((bl <= _MASK_SKIP).all() for bl in blks):
                cls[(kb, c)] = "skip"
            elif all((bl == 0.0).all() for bl in blks):
                cls[(kb, c)] = "zero"
            else:
                cls[(kb, c)] = "add"
    for c in range(NCH):
        assert any(cls[(kb, c)] != "skip" for kb in range(LT)), (
            "fully-masked query chunk: softmax undefined"
        )
    return in_maps, cls


def _build_nc(cls):
    import concourse.bass as bass
    import concourse.tile as tile
    from concourse import mybir

    f32 = mybir.dt.float32
    bf16 = mybir.dt.bfloat16
    AF = mybir.ActivationFunctionType
    ALU = mybir.AluOpType

    nc = bass.Bass("TRN2", target_bir_lowering=False, debug=False)
    seqT = nc.dram_tensor("seqT", [D, L], bf16, kind="ExternalInput").ap()
    wqT = nc.dram_tensor("wqT", [D, JW], bf16, kind="ExternalInput").ap()
    wkT = nc.dram_tensor("wkT", [D, JW], bf16, kind="ExternalInput").ap()
    wvT = nc.dram_tensor("wvT", [D, JW], bf16, kind="ExternalInput").ap()
    bq = nc.dram_tensor("bq", [JW], f32, kind="ExternalInput").ap()
    bk = nc.dram_tensor("bk", [JW], f32, kind="ExternalInput").ap()
    bv = nc.dram_tensor("bv", [JW], bf16, kind="ExternalInput").ap()
    maskT = nc.dram_tensor("maskT", [L, L], f32, kind="ExternalInput").ap()
    outT = nc.dram_tensor("outT", [JW, L], f32, kind="ExternalOutput").ap()

    def r(ap):
        return ap

    rscratch = nc.dram_tensor("rscratch", [JT, 2, NCH, CH], f32).ap()

    with tile.TileContext(nc) as tc:
        # ---- persistent pool: qT/kT/v'/out tiles (live through attention)
        with tc.tile_pool(name="psum", bufs=1, space="PSUM") as psum, tc.tile_pool(name="persist", bufs=1) as pers:
            qts = [pers.tile([P, L], bf16, tag=f"q{t}", name=f"qt{t}") for t in range(JT)]
            kts = [pers.tile([P, L], bf16, tag=f"k{t}", name=f"kt{t}") for t in range(JT)]
            v1 = [pers.tile([P, HPC * (DH + 1)], bf16, tag=f"v{t}", name=f"v1t{t}") for t in range(LT)]
            ot = [pers.tile([P, L], f32, tag=f"o{t}", name=f"ot{t}") for t in range(JT)]

            # ---- projection phase (scoped pool, freed before attention) ----
            with tc.tile_pool(name="proj", bufs=1) as proj:
                st = [proj.tile([P, L], bf16, tag=f"s{i}", name=f"st{i}") for i in range(KT)]
                for i in range(KT):
                    nc.sync.dma_start(st[i], seqT[i * P : (i + 1) * P, :])
                wq = [proj.tile([P, JW], bf16, tag=f"wq{i}", name=f"wqs{i}") for i in range(KT)]
                wk = [proj.tile([P, JW], bf16, tag=f"wk{i}", name=f"wks{i}") for i in range(KT)]
                wv = [proj.tile([P, JW], bf16, tag=f"wv{i}", name=f"wvs{i}") for i in range(KT)]
                for i in range(KT):
                    nc.sync.dma_start(wq[i], wqT[i * P : (i + 1) * P, :])
                    nc.sync.dma_start(wk[i], wkT[i * P : (i + 1) * P, :])
                    nc.sync.dma_start(wv[i], wvT[i * P : (i + 1) * P, :])
                bqt = proj.tile([P, JT], f32, tag="bq")
                bkt = proj.tile([P, JT], f32, tag="bk")
                nc.sync.dma_start(bqt, bq.rearrange("(t p) -> p t", p=P))
                nc.sync.dma_start(bkt, bk.rearrange("(t p) -> p t", p=P))
                bvr = proj.tile([1, JW], bf16, tag="bv")
                nc.sync.dma_start(bvr, bv.unsqueeze(0))
                ones1 = proj.tile([1, P], bf16, tag="ones")
                nc.vector.memset(ones1, 1.0)

                # q/k projections -> j-major qT/kT
                for W, bias, dest in ((wq, bqt, qts), (wk, bkt, kts)):
                    for t in range(JT):
                        for c in range(NCH):
                            pp = psum.tile([P, CH], f32, tag="mm", bufs=4)
                            for ki in range(KT):
                                nc.tensor.matmul(
                                    pp,
                                    r(W[ki][:, t * P : (t + 1) * P]),
                                    r(st[ki][:, c * CH : (c + 1) * CH]),
                                    start=(ki == 0),
                                    stop=(ki == KT - 1),
                                )
                            nc.scalar.activation(
                                dest[t][:, c * CH : (c + 1) * CH],
                                pp,
                                AF.Identity,
                                bias=bias[:, t : t + 1],
                            )

                # v projection -> natural-layout v' with ones columns
                for lt in range(LT):
                    v1r = v1[lt].rearrange("p (h x) -> p h x", x=DH + 1)
                    nc.vector.memset(v1r[:, :, DH : DH + 1], 1.0)
                    pp = psum.tile([P, CH], f32, tag="mm", bufs=4)
                    for ki in range(KT):
                        nc.tensor.matmul(
                            pp,
                            r(st[ki][:, lt * P : (lt + 1) * P]),
                            r(wv[ki]),
                            start=(ki == 0),
                            stop=False,
                        )
                    nc.tensor.matmul(pp, r(ones1), r(bvr), start=False, stop=True)
                    nc.vector.tensor_copy(
                        v1r[:, :, 0:DH], pp.rearrange("p (h x) -> p h x", x=DH)
                    )

            # ---- attention phase ----
            with tc.tile_pool(name="attn", bufs=1) as attn:
                mt = {}
                for key, kind in cls.items():
                    if kind == "add":
                        kb, c = key
                        m = attn.tile([P, CH], f32, tag=f"m{kb}_{c}")
                        nc.sync.dma_start(
                            m, maskT[kb * P : (kb + 1) * P, c * CH : (c + 1) * CH]
                        )
                        mt[key] = m

                contrib = {
                    c: [kb for kb in range(LT) if cls[(kb, c)] != "skip"]
                    for c in range(NCH)
                }

                for hp in range(JT):  # head pair (heads 2hp, 2hp+1)
                    E = [
                        [attn.tile([P, L], bf16, tag="E", bufs=20, name="Et") for _ in range(LT)]
                        for _ in range(2)
                    ]
                    psa = [
                        [
                            psum.tile([DH + 1, CH], f32, tag="av", bufs=4, name="psa")
                            for _ in range(NCH)
                        ]
                        for _ in range(2)
                    ]
                    for c in range(NCH):
                        csl = slice(c * CH, (c + 1) * CH)
                        for kb in contrib[c]:
                            kind = cls[(kb, c)]
                            for hh in range(2):
                                base = hh * DH
                                pp = psum.tile([P, CH], f32, tag="mm", bufs=4)
                                nc.tensor.matmul(
                                    pp,
                                    r(kts[hp][base : base + DH, kb * P : (kb + 1) * P]),
                                    r(qts[hp][base : base + DH, csl]),
                                    start=True,
                                    stop=True,
                                )
                                if kind == "add":
                                    nc.vector.tensor_tensor(
                                        pp, pp, mt[(kb, c)], ALU.add
                                    )
                                nc.scalar.activation(E[hh][kb][:, csl], pp, AF.Exp)
                            for hh in range(2):
                                h = 2 * hp + hh
                                nc.tensor.matmul(
                                    psa[hh][c],
                                    r(v1[kb][:, h * (DH + 1) : (h + 1) * (DH + 1)]),
                                    r(E[hh][kb][:, csl]),
                                    start=(kb == contrib[c][0]),
                                    stop=(kb == contrib[c][-1]),
                                )
                    # normalization + output
                    for hh in range(2):
                        base = hh * DH
                        for c in range(NCH):
                            csl = slice(c * CH, (c + 1) * CH)
                            rcp = attn.tile([1, CH], f32, tag="rcp", bufs=4)
                            nc.vector.reciprocal(rcp, psa[hh][c][DH : DH + 1, :])
                            rs = rscratch[hp, hh, c, :].unsqueeze(0)
                            nc.sync.dma_start(rs, rcp)
                            rb = attn.tile([DH, CH], f32, tag="rb", bufs=4)
                            nc.sync.dma_start(rb, rs.broadcast_to([DH, CH]))
                            nc.vector.tensor_tensor(
                                ot[hp][base : base + DH, csl],
                                psa[hh][c][0:DH, :],
                                rb,
                                ALU.mult,
                            )
                    nc.sync.dma_start(outT[hp * P : (hp + 1) * P, :], ot[hp])

    return nc


def kernel(sequence, padding_mask, dependency_mask, Wq, bq, Wk, bk, Wv, bv):
    _install_patches()
    from concourse.bass_utils import run_bass_kernel_spmd

    in_maps, cls = _prepare_inputs(
        sequence, padding_mask, dependency_mask, Wq, bq, Wk, bk, Wv, bv
    )
    nc = _build_nc(cls)
    res = run_bass_kernel_spmd(nc, in_maps, core_ids=list(range(NCORES)), trace=False)
    out = np.empty((B, L, D), np.float32)
    for c in range(NCORES):
        b, g = c // 2, c % 2
        out[b, :, g * JW : (g + 1) * JW] = res.results[c]["outT"].T
    return out


# revision 16
# speedup vs baseline: 1.2377x; 1.2377x over previous
"""Multi-head self-attention (B=4, L=1024, D=1024, H=16) on 8 TRN2 NeuronCores.

Sharding: core c handles batch b=c//2 and head-group g=c%2 (8 of 16 heads,
i.e. 512 of 1024 output columns). No collectives needed — each core computes
a disjoint [1024, 512] slice of the output.

Per-core dataflow (all layouts chosen so no on-device transpose is needed):
  - host passes seq^T [D, L], W^T slices [D, 512] (q-weights pre-scaled by
    1/sqrt(dh)), and the combined additive mask transposed: maskT[l_k, l_q]
  - qT/kT [512(j), L]: j-major projections (PE matmul, PSUM accum over D,
    bias added per-partition by ScalarE on the PSUM->SBUF copy)
  - v' [L, 8*(64+1)]: natural-layout V with a ones-column per head (the
    ones-column makes the attn@V matmul also produce the softmax denominator)
  - scoresT_h[l_k, l_q] = kT_h.T @ qT_h  (contraction over dh on partitions)
  - masked blocks are classified at build time from the actual mask values:
    all-masked blocks are skipped entirely, all-zero blocks skip the DVE add,
    partial blocks get a DVE tensor-tensor add of the mask tile
  - softmax without max-subtraction (scores are O(1) for sane inputs; the
    additive mask is clamped to -100 on host): E = exp(scoresT) via ScalarE
  - out'^T_h[dh+1, l_q] = v'_h.T @ E_h accumulated over l_k tiles; row 64 is
    the softmax denominator; DVE multiplies rows 0..63 by its broadcast
    reciprocal -> outT [512(j), L] stored j-major; host transposes on gather.

Matmul-facing tensors are bf16 (full-rate PE, FWL weight loads); all
accumulation is fp32 in PSUM, masks/normalization/output stay fp32.
"""

import sys
import types

import numpy as np

B, L, D, H = 4, 1024, 1024, 16
DH = D // H  # 64
NCORES = 8
HPC = H // 2  # heads per core
JW = HPC * DH  # 512 output cols per core
P = 128
CH = 512  # l_q chunk width
NCH = L // CH  # 2
KT = D // P  # 8 contraction tiles
LT = L // P  # 8 l tiles
JT = JW // P  # 4 j tiles
SCALE = 1.0 / np.sqrt(np.float32(DH))

_MASK_CLAMP = -100.0
_MASK_SKIP = -50.0


def _install_patches():
    """Environment glue for running bass under axon in this container."""
    # 1. antenv.axon_hooks shim so trace=True can reach the NTFF profiler.
    if "antenv.axon_hooks" not in sys.modules:
        try:
            import antenv
            from trn_agent_boot.trn_boot import _ntff_profile_via_ctypes

            hook = _ntff_profile_via_ctypes("/opt/axon/libaxon_pjrt.so")
            mod = types.ModuleType("antenv.axon_hooks")
            _h = [hook]
            mod.set_axon_ntff_profile_hook = lambda h: _h.__setitem__(0, h)
            mod.get_axon_ntff_profile_hook = lambda: _h[0]
            sys.modules["antenv.axon_hooks"] = mod
            antenv.axon_hooks = mod
        except Exception:
            pass

    # 2. no-op artifact upload (no bucket in this sandbox)
    import concourse.bass_utils as bu

    bu.upload_artifacts = lambda tmpdir: tmpdir

    # 2b. this toolchain's walrus encodes at most ONE sync-wait per
    # instruction; split extra waits onto NoOps inserted just before the
    # instruction (same engine => identical blocking semantics).
    import json

    import concourse.bass as bass_mod

    if not getattr(bass_mod.Bass, "_wait_split_patched", False):
        _orig_to_json = bass_mod.Bass.to_json_bytes

        def _split_waits_json(bir_bytes):
            j = json.loads(bir_bytes)
            ctr = 0
            for f in j["functions"]:
                for bb in f["blocks"]:
                    new = []
                    changed = False
                    for inst in bb["instructions"]:
                        si = inst.get("sync_info")
                        waits = si.get("on_wait", []) if si else []
                        if len(waits) > 1 and inst.get("engine", "Unassigned") != "Unassigned":
                            for w in waits[:-1]:
                                ctr += 1
                                new.append(
                                    {
                                        "debug": inst.get("debug", 0),
                                        "engine": inst["engine"],
                                        "ins": [],
                                        "outs": [],
                                        "name": f"I-wsplit-{ctr}",
                                        "opcode": "NoOp",
                                        "sync_info": {"on_update": [], "on_wait": [w]},
                                        "text_hint": "wait_split",
                                    }
                                )
                            si["on_wait"] = [waits[-1]]
                            changed = True
                        new.append(inst)
                    if changed:
                        bb["instructions"] = new
            return json.dumps(j).encode()

        def _to_json_bytes_split(self):
            return _split_waits_json(_orig_to_json(self))

        bass_mod.Bass.to_json_bytes = _to_json_bytes_split
        bass_mod.Bass._wait_split_patched = True

    # 3. split the Tile kernel-tail drain's multi-sem wait into single-wait
    # NOPs (this toolchain's walrus encodes only one sync-wait per CTRL).
    import concourse.tile as tile
    from concourse import mybir
    from concourse.tile import ScopedClock

    if getattr(tile.TileContext, "_drain_split_patched", False):
        return

    def _drain_and_barrier_split(self, tick_clock, wait_clock):
        nc = self.nc
        probe = nc.sync.nop(nofuse=True, hint="drain_wait_probe")
        wait_clock.add_sem_waits(
            probe.ins, ScopedClock({None: tick_clock.global_clock})
        )
        si = probe.ins.sync_info
        waits = list(si.on_wait) if si and si.on_wait else []
        if len(waits) > 1:
            si.on_wait = [waits[0]]
            for w in waits[1:]:
                extra = nc.sync.nop(nofuse=True, hint="drain_wait_split")
                xsi = extra.ins.sync_info
                if xsi is None:
                    extra.ins.sync_info = mybir.SyncInfo(on_wait=[w], on_update=[])
                else:
                    xsi.on_wait = [w]
        nc.sync.drain()
        nc.all_engine_barrier()
        assert self.sems is not None
        popped = nc._tile_sem_poison_stack.pop()
        assert popped is self._sem_poison
        nc.clear_and_free_semaphores(list(self.sems.allocated().values()))
        nc.all_engine_barrier()

    tile.TileContext._drain_and_barrier = _drain_and_barrier_split
    tile.TileContext._drain_split_patched = True


def _prepare_inputs(sequence, padding_mask, dependency_mask, Wq, bq, Wk, bk, Wv, bv):
    """Shard + pre-transpose on host. Returns (in_maps, block classification)."""
    seq = np.asarray(sequence, np.float32)
    pad = np.asarray(padding_mask, np.float32)
    dep = np.asarray(dependency_mask, np.float32)[0, 0]  # [L(q), L(k)]
    depT = np.ascontiguousarray(dep.T)  # [L(k), L(q)]
    Wq = np.asarray(Wq, np.float32)
    Wk = np.asarray(Wk, np.float32)
    Wv = np.asarray(Wv, np.float32)
    bq = np.asarray(bq, np.float32)
    bk = np.asarray(bk, np.float32)
    bv = np.asarray(bv, np.float32)

    seqT = [np.ascontiguousarray(seq[b].T) for b in range(B)]
    maskT = []
    for b in range(B):
        m = depT + pad[b, 0, 0, :][:, None]
        np.clip(m, _MASK_CLAMP, None, out=m)
        maskT.append(m)

    import ml_dtypes

    b16 = ml_dtypes.bfloat16
    in_maps = []
    for c in range(NCORES):
        b, g = c // 2, c % 2
        Jl = g * JW
        in_maps.append(
            {
                "seqT": seqT[b].astype(b16),
                "wqT": np.ascontiguousarray(Wq[Jl : Jl + JW].T * SCALE).astype(b16),
                "wkT": np.ascontiguousarray(Wk[Jl : Jl + JW].T).astype(b16),
                "wvT": np.ascontiguousarray(Wv[Jl : Jl + JW].T).astype(b16),
                "bq": np.ascontiguousarray(bq[Jl : Jl + JW] * SCALE),
                "bk": np.ascontiguousarray(bk[Jl : Jl + JW]),
                "bv": np.ascontiguousarray(bv[Jl : Jl + JW]).astype(b16),
                # multiplicative mask exp(m): E = exp(s) * exp(m)
                "expmT": np.exp(maskT[b]).astype(b16),
            }
        )

    # Block classification, shared across cores (the NEFF is SPMD): a block
    # may be skipped only if fully masked for every core; needs a DVE add
    # unless it is exactly zero for every core.
    cls = {}
    for kb in range(LT):
        for c in range(NCH):
            blks = [m[kb * P : (kb + 1) * P, c * CH : (c + 1) * CH] for m in maskT]
            if all((bl <= _MASK_SKIP).all() for bl in blks):
                cls[(kb, c)] = "skip"
            elif all((bl == 0.0).all() for bl in blks):
                cls[(kb, c)] = "zero"
            else:
                cls[(kb, c)] = "add"
    for c in range(NCH):
        assert any(cls[(kb, c)] != "skip" for kb in range(LT)), (
            "fully-masked query chunk: softmax undefined"
        )
    return in_maps, cls


def _build_nc(cls):
    import concourse.bass as bass
    import concourse.tile as tile
    from concourse import mybir

    f32 = mybir.dt.float32
    bf16 = mybir.dt.bfloat16
    AF = mybir.ActivationFunctionType
    ALU = mybir.AluOpType

    nc = bass.Bass("TRN2", target_bir_lowering=False, debug=False)
    seqT = nc.dram_tensor("seqT", [D, L], bf16, kind="ExternalInput").ap()
    wqT = nc.dram_tensor("wqT", [D, JW], bf16, kind="ExternalInput").ap()
    wkT = nc.dram_tensor("wkT", [D, JW], bf16, kind="ExternalInput").ap()
    wvT = nc.dram_tensor("wvT", [D, JW], bf16, kind="ExternalInput").ap()
    bq = nc.dram_tensor("bq", [JW], f32, kind="ExternalInput").ap()
    bk = nc.dram_tensor("bk", [JW], f32, kind="ExternalInput").ap()
    bv = nc.dram_tensor("bv", [JW], bf16, kind="ExternalInput").ap()
    expmT = nc.dram_tensor("expmT", [L, L], bf16, kind="ExternalInput").ap()
    # unnormalized output + softmax denominators: [hp, hh, c, dh|sum, l_q]
    uout = nc.dram_tensor("uout", [JT, 2, NCH, DH + 1, CH], f32, kind="ExternalOutput").ap()

    def r(ap):
        return ap

    with tile.TileContext(nc) as tc:
        # ---- persistent pool: qT/kT/v'/out tiles (live through attention)
        with tc.tile_pool(name="psum", bufs=1, space="PSUM") as psum, tc.tile_pool(name="persist", bufs=1) as pers:
            qts = [pers.tile([P, L], bf16, tag=f"q{t}", name=f"qt{t}") for t in range(JT)]
            kts = [pers.tile([P, L], bf16, tag=f"k{t}", name=f"kt{t}") for t in range(JT)]
            v1 = [pers.tile([P, HPC * (DH + 1)], bf16, tag=f"v{t}", name=f"v1t{t}") for t in range(LT)]

            # prefetch multiplicative mask tiles for partially-masked blocks
            mt = {}
            for key, kind in sorted(cls.items()):
                if kind == "add":
                    kb, c = key
                    m = pers.tile([P, CH], bf16, tag=f"m{kb}_{c}", name=f"mt{kb}_{c}")
                    nc.sync.dma_start(
                        m, expmT[kb * P : (kb + 1) * P, c * CH : (c + 1) * CH]
                    )
                    mt[key] = m

            # ---- projection phase (scoped pool, freed before attention) ----
            with tc.tile_pool(name="proj", bufs=1) as proj:
                st = [proj.tile([P, L], bf16, tag=f"s{i}", name=f"st{i}") for i in range(KT)]
                for i in range(KT):
                    nc.sync.dma_start(st[i], seqT[i * P : (i + 1) * P, :])
                wq = [proj.tile([P, JW], bf16, tag=f"wq{i}", name=f"wqs{i}") for i in range(KT)]
                wk = [proj.tile([P, JW], bf16, tag=f"wk{i}", name=f"wks{i}") for i in range(KT)]
                wv = [proj.tile([P, JW], bf16, tag=f"wv{i}", name=f"wvs{i}") for i in range(KT)]
                for i in range(KT):
                    nc.sync.dma_start(wq[i], wqT[i * P : (i + 1) * P, :])
                    nc.sync.dma_start(wk[i], wkT[i * P : (i + 1) * P, :])
                    nc.sync.dma_start(wv[i], wvT[i * P : (i + 1) * P, :])
                bqt = proj.tile([P, JT], f32, tag="bq")
                bkt = proj.tile([P, JT], f32, tag="bk")
                nc.sync.dma_start(bqt, bq.rearrange("(t p) -> p t", p=P))
                nc.sync.dma_start(bkt, bk.rearrange("(t p) -> p t", p=P))
                bvr = proj.tile([1, JW], bf16, tag="bv")
                nc.sync.dma_start(bvr, bv.unsqueeze(0))
                ones1 = proj.tile([1, P], bf16, tag="ones")
                nc.vector.memset(ones1, 1.0)

                # q/k projections -> j-major qT/kT
                for W, bias, dest in ((wq, bqt, qts), (wk, bkt, kts)):
                    for t in range(JT):
                        for c in range(NCH):
                            pp = psum.tile([P, CH], f32, tag="mm", bufs=4)
                            for ki in range(KT):
                                nc.tensor.matmul(
                                    pp,
                                    r(W[ki][:, t * P : (t + 1) * P]),
                                    r(st[ki][:, c * CH : (c + 1) * CH]),
                                    start=(ki == 0),
                                    stop=(ki == KT - 1),
                                )
                            nc.scalar.activation(
                                dest[t][:, c * CH : (c + 1) * CH],
                                pp,
                                AF.Identity,
                                bias=bias[:, t : t + 1],
                            )

                # v projection -> natural-layout v' with ones columns
                for lt in range(LT):
                    v1r = v1[lt].rearrange("p (h x) -> p h x", x=DH + 1)
                    nc.vector.memset(v1r[:, :, DH : DH + 1], 1.0)
                    pp = psum.tile([P, CH], f32, tag="mm", bufs=4)
                    for ki in range(KT):
                        nc.tensor.matmul(
                            pp,
                            r(st[ki][:, lt * P : (lt + 1) * P]),
                            r(wv[ki]),
                            start=(ki == 0),
                            stop=False,
                        )
                    nc.tensor.matmul(pp, r(ones1), r(bvr), start=False, stop=True)
                    nc.vector.tensor_copy(
                        v1r[:, :, 0:DH], pp.rearrange("p (h x) -> p h x", x=DH)
                    )

            # ---- attention phase ----
            with tc.tile_pool(name="attn", bufs=1) as attn:
                contrib = {
                    c: [kb for kb in range(LT) if cls[(kb, c)] != "skip"]
                    for c in range(NCH)
                }

                for hp in range(JT):  # head pair (heads 2hp, 2hp+1)
                    E = [
                        [attn.tile([P, L], bf16, tag="E", bufs=34, name="Et") for _ in range(LT)]
                        for _ in range(2)
                    ]
                    psa = [
                        [
                            psum.tile([DH + 1, CH], f32, tag="av", bufs=4, name="psa")
                            for _ in range(NCH)
                        ]
                        for _ in range(2)
                    ]
                    for c in range(NCH):
                        csl = slice(c * CH, (c + 1) * CH)
                        for kb in contrib[c]:
                            kind = cls[(kb, c)]
                            for hh in range(2):
                                base = hh * DH
                                pp = psum.tile([P, CH], f32, tag="mm", bufs=4)
                                nc.tensor.matmul(
                                    pp,
                                    r(kts[hp][base : base + DH, kb * P : (kb + 1) * P]),
                                    r(qts[hp][base : base + DH, csl]),
                                    start=True,
                                    stop=True,
                                )
                                nc.scalar.activation(E[hh][kb][:, csl], pp, AF.Exp)
                                if kind == "add":
                                    esl = E[hh][kb][:, csl]
                                    nc.vector.tensor_tensor(
                                        esl, esl, mt[(kb, c)], ALU.mult
                                    )
                            for hh in range(2):
                                h = 2 * hp + hh
                                nc.tensor.matmul(
                                    psa[hh][c],
                                    r(v1[kb][:, h * (DH + 1) : (h + 1) * (DH + 1)]),
                                    r(E[hh][kb][:, csl]),
                                    start=(kb == contrib[c][0]),
                                    stop=(kb == contrib[c][-1]),
                                )
                    # ship unnormalized output + denominators; divide on host
                    for hh in range(2):
                        for c in range(NCH):
                            u65 = attn.tile([DH + 1, CH], f32, tag="u65", bufs=6, name="u65")
                            nc.vector.tensor_copy(u65, psa[hh][c])
                            nc.sync.dma_start(uout[hp, hh, c], u65)

    return nc


def kernel(sequence, padding_mask, dependency_mask, Wq, bq, Wk, bk, Wv, bv):
    _install_patches()
    from concourse.bass_utils import run_bass_kernel_spmd

    in_maps, cls = _prepare_inputs(
        sequence, padding_mask, dependency_mask, Wq, bq, Wk, bk, Wv, bv
    )
    nc = _build_nc(cls)
    res = run_bass_kernel_spmd(nc, in_maps, core_ids=list(range(NCORES)), trace=False)
    out = np.empty((B, L, D), np.float32)
    for c in range(NCORES):
        b, g = c // 2, c % 2
        out[b, :, g * JW : (g + 1) * JW] = gather_core(res.results[c]["uout"])
    return out


def gather_core(uo):
    """[hp, hh, c, dh|sum, l_q] unnormalized -> [L, JW] normalized slice."""
    u = uo[:, :, :, :DH, :]  # [JT, 2, NCH, DH, CH]
    s = uo[:, :, :, DH:, :]  # [JT, 2, NCH, 1, CH]
    q = u / s
    # [hp, hh, c, dh, l] -> [l(c,ch), j(hp,hh,dh)]
    return np.ascontiguousarray(
        q.transpose(2, 4, 0, 1, 3).reshape(L, JW)
    )


# revision 20
# speedup vs baseline: 1.3956x; 1.1276x over previous
"""Multi-head self-attention (B=4, L=1024, D=1024, H=16) on 8 TRN2 NeuronCores.

Sharding: core c handles batch b=c//2 and head-group g=c%2 (8 of 16 heads,
i.e. 512 of 1024 output columns). No collectives needed — each core computes
a disjoint [1024, 512] slice of the output.

Per-core dataflow (all layouts chosen so no on-device transpose is needed):
  - host passes seq^T [D, L], W^T slices [D, 512] (q-weights pre-scaled by
    1/sqrt(dh)), and the combined additive mask transposed: maskT[l_k, l_q]
  - qT/kT [512(j), L]: j-major projections (PE matmul, PSUM accum over D,
    bias added per-partition by ScalarE on the PSUM->SBUF copy)
  - v' [L, 8*(64+1)]: natural-layout V with a ones-column per head (the
    ones-column makes the attn@V matmul also produce the softmax denominator)
  - scoresT_h[l_k, l_q] = kT_h.T @ qT_h  (contraction over dh on partitions)
  - masked blocks are classified at build time from the actual mask values:
    all-masked blocks are skipped entirely, all-zero blocks skip the DVE add,
    partial blocks get a DVE tensor-tensor add of the mask tile
  - softmax without max-subtraction (scores are O(1) for sane inputs; the
    additive mask is clamped to -100 on host): E = exp(scoresT) via ScalarE
  - out'^T_h[dh+1, l_q] = v'_h.T @ E_h accumulated over l_k tiles; row 64 is
    the softmax denominator; DVE multiplies rows 0..63 by its broadcast
    reciprocal -> outT [512(j), L] stored j-major; host transposes on gather.

Matmul-facing tensors are bf16 (full-rate PE, FWL weight loads); all
accumulation is fp32 in PSUM, masks/normalization/output stay fp32.
"""

import sys
import types

import numpy as np

B, L, D, H = 4, 1024, 1024, 16
DH = D // H  # 64
NCORES = 8
HPC = H // 2  # heads per core
JW = HPC * DH  # 512 output cols per core
P = 128
CH = 512  # l_q chunk width
NCH = L // CH  # 2
KT = D // P  # 8 contraction tiles
LT = L // P  # 8 l tiles
JT = JW // P  # 4 j tiles
SCALE = 1.0 / np.sqrt(np.float32(DH))

_MASK_CLAMP = -100.0
_MASK_SKIP = -50.0


def _install_patches():
    """Environment glue for running bass under axon in this container."""
    # 1. antenv.axon_hooks shim so trace=True can reach the NTFF profiler.
    if "antenv.axon_hooks" not in sys.modules:
        try:
            import antenv
            from trn_agent_boot.trn_boot import _ntff_profile_via_ctypes

            hook = _ntff_profile_via_ctypes("/opt/axon/libaxon_pjrt.so")
            mod = types.ModuleType("antenv.axon_hooks")
            _h = [hook]
            mod.set_axon_ntff_profile_hook = lambda h: _h.__setitem__(0, h)
            mod.get_axon_ntff_profile_hook = lambda: _h[0]
            sys.modules["antenv.axon_hooks"] = mod
            antenv.axon_hooks = mod
        except Exception:
            pass

    # 2. no-op artifact upload (no bucket in this sandbox)
    import concourse.bass_utils as bu

    bu.upload_artifacts = lambda tmpdir: tmpdir

    # 2b. this toolchain's walrus encodes at most ONE sync-wait per
    # instruction; split extra waits onto NoOps inserted just before the
    # instruction (same engine => identical blocking semantics).
    import json

    import concourse.bass as bass_mod

    if not getattr(bass_mod.Bass, "_wait_split_patched", False):
        _orig_to_json = bass_mod.Bass.to_json_bytes

        def _split_waits_json(bir_bytes):
            j = json.loads(bir_bytes)
            ctr = 0
            for f in j["functions"]:
                for bb in f["blocks"]:
                    new = []
                    changed = False
                    for inst in bb["instructions"]:
                        si = inst.get("sync_info")
                        waits = si.get("on_wait", []) if si else []
                        if len(waits) > 1 and inst.get("engine", "Unassigned") != "Unassigned":
                            for w in waits[:-1]:
                                ctr += 1
                                new.append(
                                    {
                                        "debug": inst.get("debug", 0),
                                        "engine": inst["engine"],
                                        "ins": [],
                                        "outs": [],
                                        "name": f"I-wsplit-{ctr}",
                                        "opcode": "NoOp",
                                        "sync_info": {"on_update": [], "on_wait": [w]},
                                        "text_hint": "wait_split",
                                    }
                                )
                            si["on_wait"] = [waits[-1]]
                            changed = True
                        new.append(inst)
                    if changed:
                        bb["instructions"] = new
            return json.dumps(j).encode()

        def _to_json_bytes_split(self):
            return _split_waits_json(_orig_to_json(self))

        bass_mod.Bass.to_json_bytes = _to_json_bytes_split
        bass_mod.Bass._wait_split_patched = True

    # 3. split the Tile kernel-tail drain's multi-sem wait into single-wait
    # NOPs (this toolchain's walrus encodes only one sync-wait per CTRL).
    import concourse.tile as tile
    from concourse import mybir
    from concourse.tile import ScopedClock

    if getattr(tile.TileContext, "_drain_split_patched", False):
        return

    def _drain_and_barrier_split(self, tick_clock, wait_clock):
        nc = self.nc
        probe = nc.sync.nop(nofuse=True, hint="drain_wait_probe")
        wait_clock.add_sem_waits(
            probe.ins, ScopedClock({None: tick_clock.global_clock})
        )
        si = probe.ins.sync_info
        waits = list(si.on_wait) if si and si.on_wait else []
        if len(waits) > 1:
            si.on_wait = [waits[0]]
            for w in waits[1:]:
                extra = nc.sync.nop(nofuse=True, hint="drain_wait_split")
                xsi = extra.ins.sync_info
                if xsi is None:
                    extra.ins.sync_info = mybir.SyncInfo(on_wait=[w], on_update=[])
                else:
                    xsi.on_wait = [w]
        nc.sync.drain()
        nc.all_engine_barrier()
        assert self.sems is not None
        popped = nc._tile_sem_poison_stack.pop()
        assert popped is self._sem_poison
        nc.clear_and_free_semaphores(list(self.sems.allocated().values()))
        nc.all_engine_barrier()

    tile.TileContext._drain_and_barrier = _drain_and_barrier_split
    tile.TileContext._drain_split_patched = True


def _prepare_inputs(sequence, padding_mask, dependency_mask, Wq, bq, Wk, bk, Wv, bv):
    """Shard + pre-transpose on host. Returns (in_maps, block classification)."""
    seq = np.asarray(sequence, np.float32)
    pad = np.asarray(padding_mask, np.float32)
    dep = np.asarray(dependency_mask, np.float32)[0, 0]  # [L(q), L(k)]
    depT = np.ascontiguousarray(dep.T)  # [L(k), L(q)]
    Wq = np.asarray(Wq, np.float32)
    Wk = np.asarray(Wk, np.float32)
    Wv = np.asarray(Wv, np.float32)
    bq = np.asarray(bq, np.float32)
    bk = np.asarray(bk, np.float32)
    bv = np.asarray(bv, np.float32)

    seqT = [np.ascontiguousarray(seq[b].T) for b in range(B)]
    maskT = []
    for b in range(B):
        m = depT + pad[b, 0, 0, :][:, None]
        np.clip(m, _MASK_CLAMP, None, out=m)
        maskT.append(m)

    import ml_dtypes

    b16 = ml_dtypes.bfloat16
    in_maps = []
    for c in range(NCORES):
        b, g = c // 2, c % 2
        Jl = g * JW
        in_maps.append(
            {
                "seqT": seqT[b].astype(b16),
                "wqT": np.ascontiguousarray(Wq[Jl : Jl + JW].T * SCALE).astype(b16),
                "wkT": np.ascontiguousarray(Wk[Jl : Jl + JW].T).astype(b16),
                "wvT": np.ascontiguousarray(Wv[Jl : Jl + JW].T).astype(b16),
                "bq": np.ascontiguousarray(bq[Jl : Jl + JW] * SCALE),
                "bk": np.ascontiguousarray(bk[Jl : Jl + JW]),
                "bv": np.ascontiguousarray(bv[Jl : Jl + JW]).astype(b16),
                # multiplicative mask exp(m): E = exp(s) * exp(m)
                "expmT": np.exp(maskT[b]).astype(b16),
            }
        )

    # Column-restriction classification at 128-col granularity, shared
    # across cores (the NEFF is SPMD, so decisions must hold for every core).
    # Per l_k row-block kb:
    #   vs[kb]   - first l_q col (128-aligned) not fully masked (None: drop kb)
    #   mrng[kb] - [ps, pe) col range needing the multiplicative mask (covers
    #              'add' sub-blocks and any interior fully-masked sub-blocks)
    SUB = 128
    NSUB = L // SUB
    vs = {}
    mrng = {}
    for kb in range(LT):
        sub = []
        for cb in range(NSUB):
            blks = [m[kb * P : (kb + 1) * P, cb * SUB : (cb + 1) * SUB] for m in maskT]
            if all((bl <= _MASK_SKIP).all() for bl in blks):
                sub.append("skip")
            elif all((bl == 0.0).all() for bl in blks):
                sub.append("zero")
            else:
                sub.append("add")
        keep = [cb for cb in range(NSUB) if sub[cb] != "skip"]
        if not keep:
            vs[kb] = None
            mrng[kb] = None
            continue
        v0 = keep[0]
        vs[kb] = v0 * SUB
        need = [cb for cb in range(v0, NSUB) if sub[cb] != "zero"]
        mrng[kb] = (need[0] * SUB, (need[-1] + 1) * SUB) if need else None
    for c in range(NCH):
        ks = [kb for kb in range(LT) if vs[kb] is not None and vs[kb] < (c + 1) * CH]
        assert ks and min(vs[kb] for kb in ks) <= c * CH, (
            "degenerate mask: some query columns are fully masked"
        )
    cls = {"vs": vs, "mrng": mrng}
    return in_maps, cls


def _build_nc(cls):
    import concourse.bass as bass
    import concourse.tile as tile
    from concourse import mybir

    f32 = mybir.dt.float32
    bf16 = mybir.dt.bfloat16
    AF = mybir.ActivationFunctionType
    ALU = mybir.AluOpType

    nc = bass.Bass("TRN2", target_bir_lowering=False, debug=False)
    seqT = nc.dram_tensor("seqT", [D, L], bf16, kind="ExternalInput").ap()
    wqT = nc.dram_tensor("wqT", [D, JW], bf16, kind="ExternalInput").ap()
    wkT = nc.dram_tensor("wkT", [D, JW], bf16, kind="ExternalInput").ap()
    wvT = nc.dram_tensor("wvT", [D, JW], bf16, kind="ExternalInput").ap()
    bq = nc.dram_tensor("bq", [JW], f32, kind="ExternalInput").ap()
    bk = nc.dram_tensor("bk", [JW], f32, kind="ExternalInput").ap()
    bv = nc.dram_tensor("bv", [JW], bf16, kind="ExternalInput").ap()
    expmT = nc.dram_tensor("expmT", [L, L], bf16, kind="ExternalInput").ap()
    # unnormalized output + softmax denominators: [hp, hh, c, dh|sum, l_q]
    uout = nc.dram_tensor("uout", [JT, 2, NCH, DH + 1, CH], f32, kind="ExternalOutput").ap()

    def r(ap):
        return ap

    with tile.TileContext(nc) as tc:
        # ---- persistent pool: qT/kT/v'/out tiles (live through attention)
        with tc.tile_pool(name="psum", bufs=1, space="PSUM") as psum, tc.tile_pool(name="persist", bufs=1) as pers:
            qts = [pers.tile([P, L], bf16, tag=f"q{t}", name=f"qt{t}") for t in range(JT)]
            kts = [pers.tile([P, L], bf16, tag=f"k{t}", name=f"kt{t}") for t in range(JT)]
            v1 = [pers.tile([P, HPC * (DH + 1)], bf16, tag=f"v{t}", name=f"v1t{t}") for t in range(LT)]

            # prefetch multiplicative mask tiles for partially-masked ranges
            vs, mrng = cls["vs"], cls["mrng"]
            mt = {}
            for kb in range(LT):
                if vs[kb] is None or mrng[kb] is None:
                    continue
                ps, pe = mrng[kb]
                m = pers.tile([P, pe - ps], bf16, tag=f"m{kb}", name=f"mt{kb}")
                nc.sync.dma_start(m, expmT[kb * P : (kb + 1) * P, ps:pe])
                mt[kb] = m

            # ---- projection phase (scoped pool, freed before attention) ----
            with tc.tile_pool(name="proj", bufs=1) as proj:
                st = [proj.tile([P, L], bf16, tag=f"s{i}", name=f"st{i}") for i in range(KT)]
                wq = [proj.tile([P, JW], bf16, tag=f"wq{i}", name=f"wqs{i}") for i in range(KT)]
                wk = [proj.tile([P, JW], bf16, tag=f"wk{i}", name=f"wks{i}") for i in range(KT)]
                wv = [proj.tile([P, JW], bf16, tag=f"wv{i}", name=f"wvs{i}") for i in range(KT)]
                for i in range(KT):
                    nc.sync.dma_start(st[i], seqT[i * P : (i + 1) * P, :])
                    nc.sync.dma_start(wq[i], wqT[i * P : (i + 1) * P, :])
                for i in range(KT):
                    nc.sync.dma_start(wk[i], wkT[i * P : (i + 1) * P, :])
                for i in range(KT):
                    nc.sync.dma_start(wv[i], wvT[i * P : (i + 1) * P, :])
                bqt = proj.tile([P, JT], f32, tag="bq")
                bkt = proj.tile([P, JT], f32, tag="bk")
                nc.sync.dma_start(bqt, bq.rearrange("(t p) -> p t", p=P))
                nc.sync.dma_start(bkt, bk.rearrange("(t p) -> p t", p=P))
                bvr = proj.tile([1, JW], bf16, tag="bv")
                nc.sync.dma_start(bvr, bv.unsqueeze(0))
                ones1 = proj.tile([1, P], bf16, tag="ones")
                nc.vector.memset(ones1, 1.0)

                # q/k projections -> j-major qT/kT
                for W, bias, dest in ((wq, bqt, qts), (wk, bkt, kts)):
                    for t in range(JT):
                        for c in range(NCH):
                            pp = psum.tile([P, CH], f32, tag="mm", bufs=3)
                            for ki in range(KT):
                                nc.tensor.matmul(
                                    pp,
                                    r(W[ki][:, t * P : (t + 1) * P]),
                                    r(st[ki][:, c * CH : (c + 1) * CH]),
                                    start=(ki == 0),
                                    stop=(ki == KT - 1),
                                )
                            nc.vector.tensor_scalar_add(
                                dest[t][:, c * CH : (c + 1) * CH],
                                pp,
                                bias[:, t : t + 1],
                            )

                # v projection -> natural-layout v' with ones columns
                for lt in range(LT):
                    v1r = v1[lt].rearrange("p (h x) -> p h x", x=DH + 1)
                    nc.vector.memset(v1r[:, :, DH : DH + 1], 1.0)
                    pp = psum.tile([P, CH], f32, tag="mm", bufs=3)
                    for ki in range(KT):
                        nc.tensor.matmul(
                            pp,
                            r(st[ki][:, lt * P : (lt + 1) * P]),
                            r(wv[ki]),
                            start=(ki == 0),
                            stop=False,
                        )
                    nc.tensor.matmul(pp, r(ones1), r(bvr), start=False, stop=True)
                    nc.vector.tensor_copy(
                        v1r[:, :, 0:DH], pp.rearrange("p (h x) -> p h x", x=DH)
                    )

            # ---- attention phase: one head at a time, software-pipelined ----
            with tc.tile_pool(name="attn", bufs=1) as attn:
                AF_ = AF
                kbs = [kb for kb in range(LT) if vs[kb] is not None]
                contrib = {
                    c: [kb for kb in kbs if vs[kb] < (c + 1) * CH] for c in range(NCH)
                }

                for h in range(HPC):
                    hp, hh = h // 2, h % 2
                    base = hh * DH
                    E = {
                        kb: attn.tile([P, L], bf16, tag="E", bufs=18, name="Et")
                        for kb in kbs
                    }
                    psa = [
                        psum.tile([DH + 1, CH], f32, tag="av", bufs=2, name="psa")
                        for _ in range(NCH)
                    ]

                    def emit_scores(kb):
                        pp = psum.tile([P, L], f32, tag="mm", bufs=3, name="pp")
                        for c in range(NCH):
                            lo = max(c * CH, vs[kb])
                            if lo >= (c + 1) * CH:
                                continue
                            nc.tensor.matmul(
                                pp[:, lo : (c + 1) * CH],
                                kts[hp][base : base + DH, kb * P : (kb + 1) * P],
                                qts[hp][base : base + DH, lo : (c + 1) * CH],
                                start=True,
                                stop=True,
                            )
                        nc.scalar.activation(
                            E[kb][:, vs[kb] :], pp[:, vs[kb] :], AF_.Exp
                        )
                        if mrng[kb] is not None:
                            ps, pe = mrng[kb]
                            esl = E[kb][:, ps:pe]
                            nc.vector.tensor_tensor(esl, esl, mt[kb], ALU.mult)

                    def emit_av(kb):
                        for c in range(NCH):
                            if kb not in contrib[c]:
                                continue
                            lo = max(c * CH, vs[kb])
                            h65 = slice(h * (DH + 1), (h + 1) * (DH + 1))
                            nc.tensor.matmul(
                                psa[c][:, lo - c * CH :],
                                v1[kb][:, h65],
                                E[kb][:, lo : (c + 1) * CH],
                                start=(kb == contrib[c][0]),
                                stop=(kb == contrib[c][-1]),
                            )

                    LAG = 2
                    for idx, kb in enumerate(kbs):
                        emit_scores(kb)
                        if idx >= LAG:
                            emit_av(kbs[idx - LAG])
                    for kb in kbs[-LAG:]:
                        emit_av(kb)

                    # ship unnormalized output + denominators; divide on host
                    for c in range(NCH):
                        u65 = attn.tile(
                            [DH + 1, CH], f32, tag="u65", bufs=4, name="u65"
                        )
                        nc.vector.tensor_copy(u65, psa[c])
                        nc.sync.dma_start(uout[hp, hh, c], u65)

    return nc


def kernel(sequence, padding_mask, dependency_mask, Wq, bq, Wk, bk, Wv, bv):
    _install_patches()
    from concourse.bass_utils import run_bass_kernel_spmd

    in_maps, cls = _prepare_inputs(
        sequence, padding_mask, dependency_mask, Wq, bq, Wk, bk, Wv, bv
    )
    nc = _build_nc(cls)
    res = run_bass_kernel_spmd(nc, in_maps, core_ids=list(range(NCORES)), trace=False)
    out = np.empty((B, L, D), np.float32)
    for c in range(NCORES):
        b, g = c // 2, c % 2
        out[b, :, g * JW : (g + 1) * JW] = gather_core(res.results[c]["uout"])
    return out


def gather_core(uo):
    """[hp, hh, c, dh|sum, l_q] unnormalized -> [L, JW] normalized slice."""
    u = uo[:, :, :, :DH, :]  # [JT, 2, NCH, DH, CH]
    s = uo[:, :, :, DH:, :]  # [JT, 2, NCH, 1, CH]
    q = u / s
    # [hp, hh, c, dh, l] -> [l(c,ch), j(hp,hh,dh)]
    return np.ascontiguousarray(
        q.transpose(2, 4, 0, 1, 3).reshape(L, JW)
    )
